# revision 38
# baseline (speedup 1.0000x reference)
"""EpisodicMemory retrieval kernel for 8 Trainium2 NeuronCores.

Distributed KNN with minimal host->device traffic: the store is sharded
across the 8 cores (no replicated full-store input). Per core: keys-norms
via 2-pass quadratic form (||k_n||^2 ~= shi*G*shi + 2*slo*G*shi,
G = Wk^T Wk computed on-device from an AllGathered Wk), sims via 3-pass
bf16 hi/lo split matmuls, local top-8 via DVE max8; AllGather of 8*8
candidates per query; replicated global top-8 select + softmax on every
core; owner-computes partial combine (masked indirect row gather from the
local store shard) accumulated into [B, H]; ReduceScatter(add) down to the
query shard; Wv/Wo projection (vals never materialized since
softmax(s)@(store@Wv.T)@Wo.T = ((softmax(s)@store)@Wv.T)@Wo.T).
Weights Wk/Wv/Wo arrive as 128-row shards and are AllGathered on device;
the global weight-sum S is an AllReduce of local partial sums, with 1/S
folded into the softmax scale (it cannot change top-k order).
"""

import numpy as np

import concourse.bacc as bacc
import concourse.bass as bass
import concourse.mybir as mybir
from concourse.tile import TileContext
from concourse.bass_utils import run_bass_kernel_spmd
from concourse.masks import make_identity

F32 = mybir.dt.float32
BF16 = mybir.dt.bfloat16
U32 = mybir.dt.uint32
AL = mybir.AluOpType
ACTF = mybir.ActivationFunctionType

TOP_K = 8
RECENCY_DECAY = 0.99
CURRENT_TS = 1.0
BIG = 1.0e6


def build_kernel(B=2048, N=65536, H=1024, NC=8, coll=True, phase_stop="all"):
    NL = N // NC          # local store rows per core
    BSH = B // NC         # query shard per core
    WSH = H // NC         # weight row shard per core (128)
    IT = H // 128         # i-tiles (contraction chunks)
    BT = B // 128         # query tiles
    QT = BSH // 128       # query-shard tiles
    CH = 512              # n-chunk width
    NCH = NL // CH        # chunks per core
    NTC = CH // 128       # n-tiles per chunk
    assert BSH % 128 == 0 and NL % CH == 0 and H % 128 == 0 and WSH == 128

    nc = bacc.Bacc("TRN2", target_bir_lowering=False, debug=False, num_devices=NC)

    store_l = nc.dram_tensor("store_l", [NL, H], F32, kind="ExternalInput")
    imp_l = nc.dram_tensor("imp_l", [NL], F32, kind="ExternalInput")
    ts_l = nc.dram_tensor("ts_l", [NL], F32, kind="ExternalInput")
    q_sh = nc.dram_tensor("q_sh", [BSH, H], F32, kind="ExternalInput")
    wk_sh = nc.dram_tensor("wk_sh", [WSH, H], F32, kind="ExternalInput")
    wv_sh = nc.dram_tensor("wv_sh", [WSH, H], F32, kind="ExternalInput")
    wo_sh = nc.dram_tensor("wo_sh", [WSH, H], F32, kind="ExternalInput")
    nbase_d = nc.dram_tensor("nbase_d", [1, 1], F32, kind="ExternalInput")
    out_d = nc.dram_tensor("out_shard", [BSH, H], F32, kind="ExternalOutput")

    dec = 1.0 - RECENCY_DECAY
    AS = "Shared" if coll else "Local"

    with TileContext(nc) as tc:
        with (
            tc.tile_pool(name="const", bufs=1) as cst,
            tc.tile_pool(name="persist", bufs=1) as per,
            tc.tile_pool(name="dram", bufs=1, space="DRAM") as dram,
        ):
            ident = cst.tile([128, 128], F32, tag="ident", name="ident")
            make_identity(nc, ident[:])
            ones_row = cst.tile([1, 128], F32, tag="ones_row", name="ones_row")
            nc.vector.memset(ones_row[:], 1.0)
            ones_col = cst.tile([128, 1], F32, tag="ones_col", name="ones_col")
            nc.vector.memset(ones_col[:], 1.0)

            nbase_t = cst.tile([1, 1], F32, tag="nbase_t", name="nbase_t")
            nc.sync.dma_start(nbase_t[:], nbase_d[:])
            nbase_bc = cst.tile([128, 1], F32, tag="nbase_bc", name="nbase_bc")
            nc.gpsimd.partition_broadcast(nbase_bc[:], nbase_t[:])
            negdec = cst.tile([128, 1], F32, tag="negdec", name="negdec")
            nc.vector.memset(negdec[:], -dec * CURRENT_TS)

            # DRAM scratch for collectives (collective inputs must be
            # Internal tensors — the BIR verifier rejects ExternalInput
            # sources — so weight shards are staged via DRAM->DRAM DMA)
            wk_ag_in = dram.tile([WSH, H], F32, tag="wk_ag_in", name="wk_ag_in")
            wv_ag_in = dram.tile([WSH, H], F32, tag="wv_ag_in", name="wv_ag_in")
            wo_ag_in = dram.tile([WSH, H], F32, tag="wo_ag_in", name="wo_ag_in")
            wk_full = dram.tile([H, H], F32, tag="wk_full", name="wk_full", addr_space=AS)
            wv_full = dram.tile([H, H], F32, tag="wv_full", name="wv_full", addr_space=AS)
            wo_full = dram.tile([H, H], F32, tag="wo_full", name="wo_full", addr_space=AS)
            ssum_in = dram.tile([1, 1], F32, tag="ssum_in", name="ssum_in")
            ssum_out = dram.tile([1, 1], F32, tag="ssum_out", name="ssum_out", addr_space=AS)
            rq_ag_in = dram.tile([BSH, 1], F32, tag="rq_ag_in", name="rq_ag_in")
            rq_ag_out = dram.tile([B, 1], F32, tag="rq_ag_out", name="rq_ag_out", addr_space=AS)
            pack_in = dram.tile([B, 16], F32, tag="pack_in", name="pack_in")
            pack_out = dram.tile([NC * B, 16], F32, tag="pack_out", name="pack_out", addr_space=AS)
            rs_in = dram.tile([B, H], F32, tag="rs_in", name="rs_in")
            rs_out = dram.tile([BSH, H], F32, tag="rs_out", name="rs_out")

            grp = [list(range(NC))]

            # ---- weight AllGathers (issue first: inputs are ready at t=0) ----
            nc.sync.dma_start(wk_ag_in[:], wk_sh[:])
            nc.sync.dma_start(wv_ag_in[:], wv_sh[:])
            nc.sync.dma_start(wo_ag_in[:], wo_sh[:])
            if coll:
                nc.gpsimd.collective_compute(
                    "AllGather", AL.bypass, replica_groups=grp,
                    ins=[wk_ag_in.opt()], outs=[wk_full.opt()])
            else:
                for c in range(NC):
                    nc.sync.dma_start(wk_full[c * WSH:(c + 1) * WSH, :], wk_ag_in[:])

            # ---- local weights w2[p, t] = rec*(imp+1)  (n = t*128+p) ----
            NFL = NL // 128
            w2 = per.tile([128, NFL], F32, tag="w2", name="w2")
            with (
                tc.tile_pool(name="wloc", bufs=1) as wlp,
                tc.tile_pool(name="ps0", bufs=1, space="PSUM") as ps0,
            ):
                tsl_t = wlp.tile([128, NFL], F32, tag="tsl_t", name="tsl_t")
                nc.sync.dma_start(tsl_t[:], ts_l[:].rearrange("(t p) -> p t", p=128))
                impl_t = wlp.tile([128, NFL], F32, tag="impl_t", name="impl_t")
                nc.sync.dma_start(impl_t[:], imp_l[:].rearrange("(t p) -> p t", p=128))
                recl = wlp.tile([128, NFL], F32, tag="recl", name="recl")
                nc.scalar.activation(recl[:], tsl_t[:], ACTF.Exp,
                                     bias=negdec[:, 0:1], scale=dec)
                nc.vector.tensor_scalar(out=w2[:], in0=impl_t[:], scalar1=1.0,
                                        scalar2=None, op0=AL.add)
                nc.vector.tensor_tensor(out=w2[:], in0=w2[:], in1=recl[:], op=AL.mult)
                # local partial sum -> AllReduce for global S
                wsum_p = wlp.tile([128, 1], F32, tag="wsum_p", name="wsum_p")
                nc.vector.tensor_reduce(out=wsum_p[:], in_=w2[:],
                                        axis=mybir.AxisListType.X, op=AL.add)
                s_ps = ps0.tile([1, 1], F32, tag="s_ps", name="s_ps")
                nc.tensor.matmul(s_ps[:], ones_col[:], wsum_p[:], start=True, stop=True)
                s_sb = wlp.tile([1, 1], F32, tag="s_sb", name="s_sb")
                nc.scalar.copy(s_sb[:], s_ps[:])
                nc.sync.dma_start(ssum_in[:], s_sb[:])

            if coll:
                nc.gpsimd.collective_compute(
                    "AllReduce", AL.add, replica_groups=grp,
                    ins=[ssum_in.opt()], outs=[ssum_out.opt()])
            else:
                nc.sync.dma_start(ssum_out[:], ssum_in[:])

            # rv = 1 / (S + 1e-8), broadcast to partitions
            rv_bc = cst.tile([128, 1], F32, tag="rv_bc", name="rv_bc")
            with tc.tile_pool(name="rvp", bufs=1) as rvp:
                s_t = rvp.tile([1, 1], F32, tag="s_t", name="s_t")
                nc.sync.dma_start(s_t[:], ssum_out[:])
                s_t2 = rvp.tile([1, 1], F32, tag="s_t2", name="s_t2")
                nc.vector.tensor_scalar(out=s_t2[:], in0=s_t[:], scalar1=1e-8,
                                        scalar2=None, op0=AL.add)
                rv_t = rvp.tile([1, 1], F32, tag="rv_t", name="rv_t")
                nc.vector.reciprocal(rv_t[:], s_t2[:])
                nc.gpsimd.partition_broadcast(rv_bc[:], rv_t[:])

            rq = [per.tile([128, 1], F32, tag=f"rq{t}", name=f"rq{t}") for t in range(QT)]
            rq_bt = per.tile([128, BT], F32, tag="rq_bt", name="rq_bt")

            # ================= main scope =================
            with tc.tile_pool(name="mainsb", bufs=1) as msb:
                qkT_hi = [msb.tile([128, B], BF16, tag=f"qkT_hi{t}", name=f"qkT_hi{t}") for t in range(IT)]
                qkT_lo = [msb.tile([128, B], BF16, tag=f"qkT_lo{t}", name=f"qkT_lo{t}") for t in range(IT)]
                g_hi = [msb.tile([128, H], BF16, tag=f"g_hi{t}", name=f"g_hi{t}") for t in range(IT)]
                g_lo = [msb.tile([128, H], BF16, tag=f"g_lo{t}", name=f"g_lo{t}") for t in range(IT)]
                vals_all = [msb.tile([128, NCH * 8], F32, tag=f"vals_all{t}", name=f"vals_all{t}")
                            for t in range(BT)]
                idx_all = [msb.tile([128, NCH * 8], F32, tag=f"idx_all{t}", name=f"idx_all{t}")
                           for t in range(BT)]

                qkT_ag_in = dram.tile([H, BSH], F32, tag="qkT_ag_in", name="qkT_ag_in")
                qkT_ag_out = dram.tile([NC * H, BSH], F32, tag="qkT_ag_out", name="qkT_ag_out", addr_space=AS)

                # ---- prologue: q load/norms, Wk load+split, qkT shard, G ----
                with (
                    tc.tile_pool(name="prolog", bufs=1) as prl,
                    tc.tile_pool(name="ptmp", bufs=3) as ptmp,
                    tc.tile_pool(name="psP", bufs=2, space="PSUM") as psP,
                ):
                    qT_hi = [prl.tile([128, BSH], BF16, tag=f"qT_hi{t}", name=f"qT_hi{t}") for t in range(IT)]
                    qT_lo = [prl.tile([128, BSH], BF16, tag=f"qT_lo{t}", name=f"qT_lo{t}") for t in range(IT)]
                    for qt in range(QT):
                        qnat = ptmp.tile([128, H], F32, tag="qnat", name="qnat")
                        nc.sync.dma_start(qnat[:], q_sh[qt * 128:(qt + 1) * 128, :])
                        scr = ptmp.tile([128, H], F32, tag="qscr", name="qscr")
                        qn2 = ptmp.tile([128, 1], F32, tag="qn2", name="qn2")
                        nc.vector.scalar_tensor_tensor(out=scr[:], in0=qnat[:],
                                                       scalar=1.0, in1=qnat[:],
                                                       op0=AL.mult, op1=AL.mult,
                                                       accum_out=qn2[:])
                        qrec = ptmp.tile([128, 1], F32, tag="qrec", name="qrec")
                        nc.vector.reciprocal(qrec[:], qn2[:])
                        nc.scalar.sqrt(rq[qt][:], qrec[:])
                        nc.sync.dma_start(rq_ag_in[qt * 128:(qt + 1) * 128, :],
                                          rq[qt][:])
                        for it in range(IT):
                            qtp = psP.tile([128, 128], F32, tag="qtp", name="qtp")
                            nc.tensor.transpose(
                                qtp[:], qnat[:, it * 128:(it + 1) * 128], ident[:])
                            dst_hi = qT_hi[it][:, qt * 128:(qt + 1) * 128]
                            dst_lo = qT_lo[it][:, qt * 128:(qt + 1) * 128]
                            nc.scalar.copy(dst_hi, qtp[:])
                            nc.vector.tensor_tensor(out=dst_lo, in0=qtp[:], in1=dst_hi,
                                                    op=AL.subtract)

                    if coll:
                        nc.gpsimd.collective_compute(
                            "AllGather", AL.bypass, replica_groups=grp,
                            ins=[rq_ag_in.opt()], outs=[rq_ag_out.opt()])
                    else:
                        for c in range(NC):
                            nc.sync.dma_start(
                                rq_ag_out[c * BSH:(c + 1) * BSH, :], rq_ag_in[:])

                    # full Wk from AllGather -> hi/lo split tiles
                    wk_hi = [prl.tile([128, H], BF16, tag=f"wk_hi{t}", name=f"wk_hi{t}") for t in range(IT)]
                    wk_lo = [prl.tile([128, H], BF16, tag=f"wk_lo{t}", name=f"wk_lo{t}") for t in range(IT)]
                    for t in range(IT):
                        wkt = ptmp.tile([128, H], F32, tag="wkt", name="wkt")
                        nc.sync.dma_start(wkt[:], wk_full[t * 128:(t + 1) * 128, :])
                        nc.scalar.copy(wk_hi[t][:], wkt[:])
                        nc.vector.tensor_tensor(out=wk_lo[t][:], in0=wkt[:],
                                                in1=wk_hi[t][:], op=AL.subtract)

                    # qkT shard [H, BSH] = Wk^T @ q_sh^T
                    for it in range(IT):
                        qk_ps = psP.tile([128, BSH], F32, tag="qk_ps", name="qk_ps")
                        for ot in range(IT):
                            lhs_hi = wk_hi[ot][:, it * 128:(it + 1) * 128]
                            lhs_lo = wk_lo[ot][:, it * 128:(it + 1) * 128]
                            nc.tensor.matmul(qk_ps[:], lhs_hi, qT_hi[ot][:],
                                             start=(ot == 0), stop=False)
                            nc.tensor.matmul(qk_ps[:], lhs_hi, qT_lo[ot][:],
                                             start=False, stop=False)
                            nc.tensor.matmul(qk_ps[:], lhs_lo, qT_hi[ot][:],
                                             start=False, stop=(ot == IT - 1))
                        qk_sb = ptmp.tile([128, BSH], F32, tag="qk_sb", name="qk_sb")
                        nc.scalar.copy(qk_sb[:], qk_ps[:])
                        nc.sync.dma_start(qkT_ag_in[it * 128:(it + 1) * 128, :],
                                          qk_sb[:])

                    if coll:
                        nc.gpsimd.collective_compute(
                            "AllGather", AL.bypass, replica_groups=grp,
                            ins=[qkT_ag_in.opt()], outs=[qkT_ag_out.opt()])
                        nc.gpsimd.collective_compute(
                            "AllGather", AL.bypass, replica_groups=grp,
                            ins=[wv_ag_in.opt()], outs=[wv_full.opt()])
                        nc.gpsimd.collective_compute(
                            "AllGather", AL.bypass, replica_groups=grp,
                            ins=[wo_ag_in.opt()], outs=[wo_full.opt()])
                    else:
                        for c in range(NC):
                            nc.sync.dma_start(qkT_ag_out[c * H:(c + 1) * H, :], qkT_ag_in[:])
                            nc.sync.dma_start(wv_full[c * WSH:(c + 1) * WSH, :], wv_ag_in[:])
                            nc.sync.dma_start(wo_full[c * WSH:(c + 1) * WSH, :], wo_ag_in[:])

                    # G = Wk^T @ Wk, all row-blocks computed locally
                    for jb in range(IT):
                        for ih in range(H // 512):
                            g_ps = psP.tile([128, 512], F32, tag="g_ps", name="g_ps")
                            for ot in range(IT):
                                lhs_hi = wk_hi[ot][:, jb * 128:(jb + 1) * 128]
                                lhs_lo = wk_lo[ot][:, jb * 128:(jb + 1) * 128]
                                rhs_hi = wk_hi[ot][:, ih * 512:(ih + 1) * 512]
                                rhs_lo = wk_lo[ot][:, ih * 512:(ih + 1) * 512]
                                nc.tensor.matmul(g_ps[:], lhs_hi, rhs_hi,
                                                 start=(ot == 0), stop=False)
                                nc.tensor.matmul(g_ps[:], lhs_hi, rhs_lo,
                                                 start=False, stop=False)
                                nc.tensor.matmul(g_ps[:], lhs_lo, rhs_hi,
                                                 start=False, stop=(ot == IT - 1))
                            dst_hi = g_hi[jb][:, ih * 512:(ih + 1) * 512]
                            dst_lo = g_lo[jb][:, ih * 512:(ih + 1) * 512]
                            nc.scalar.copy(dst_hi, g_ps[:])
                            nc.vector.tensor_tensor(out=dst_lo, in0=g_ps[:],
                                                    in1=dst_hi, op=AL.subtract)

                # read back AllGathered qkT -> [128, B] hi/lo tiles; rq_bt
                with tc.tile_pool(name="rb", bufs=3) as rb:
                    for c in range(NC):
                        for it in range(IT):
                            blk = rb.tile([128, BSH], F32, tag="qkblk", name="qkblk")
                            nc.sync.dma_start(
                                blk[:],
                                qkT_ag_out[c * H + it * 128:c * H + (it + 1) * 128, :])
                            dhi = qkT_hi[it][:, c * BSH:(c + 1) * BSH]
                            dlo = qkT_lo[it][:, c * BSH:(c + 1) * BSH]
                            nc.scalar.copy(dhi, blk[:])
                            nc.vector.tensor_tensor(out=dlo, in0=blk[:], in1=dhi,
                                                    op=AL.subtract)
                    for bt in range(BT):
                        nc.sync.dma_start(rq_bt[:, bt:bt + 1],
                                          rq_ag_out[bt * 128:(bt + 1) * 128, :])

                # ---- per-chunk: split, transpose, norms, sims, chunk top-8 ----
                shi_dr = [dram.tile([CH, H], BF16, tag=f"shi_dr{j}", name=f"shi_dr{j}") for j in range(NCH)]
                slo_dr = [dram.tile([CH, H], BF16, tag=f"slo_dr{j}", name=f"slo_dr{j}") for j in range(NCH)]

                with (
                    tc.tile_pool(name="stld", bufs=2) as stld,
                    tc.tile_pool(name="split", bufs=6) as spl,
                    tc.tile_pool(name="strT", bufs=3) as strT,
                    tc.tile_pool(name="nrm", bufs=2) as nrm,
                    tc.tile_pool(name="simb", bufs=2) as simb,
                    tc.tile_pool(name="psz", bufs=1, space="PSUM") as psz,
                    tc.tile_pool(name="pssim", bufs=3, space="PSUM") as pssim,
                    tc.tile_pool(name="psmisc", bufs=1, space="PSUM") as psmisc,
                ):
                    for j in range(0 if phase_stop == "prolog" else NCH):
                        shi_nat = []
                        slo_nat = []
                        for ntl in range(NTC):
                            t = j * NTC + ntl
                            snat = stld.tile([128, H], F32, tag="snat", name="snat")
                            nc.sync.dma_start(snat[:],
                                              store_l[t * 128:(t + 1) * 128, :])
                            hi = spl.tile([128, H], BF16, tag="hi", name="hi")
                            lo = spl.tile([128, H], BF16, tag="lo", name="lo")
                            nc.scalar.copy(hi[:], snat[:])
                            nc.vector.tensor_tensor(out=lo[:], in0=snat[:], in1=hi[:],
                                                    op=AL.subtract)
                            nc.sync.dma_start(
                                shi_dr[j][ntl * 128:(ntl + 1) * 128, :], hi[:])
                            nc.sync.dma_start(
                                slo_dr[j][ntl * 128:(ntl + 1) * 128, :], lo[:])
                            shi_nat.append(hi)
                            slo_nat.append(lo)

                        sThi = []
                        sTlo = []
                        for it in range(IT):
                            th = strT.tile([128, CH], BF16, tag=f"sThi{it}", name=f"sThi{it}")
                            nc.sync.dma_start_transpose(
                                th[:], shi_dr[j][:, it * 128:(it + 1) * 128])
                            sThi.append(th)
                            tl = strT.tile([128, CH], BF16, tag=f"sTlo{it}", name=f"sTlo{it}")
                            nc.sync.dma_start_transpose(
                                tl[:], slo_dr[j][:, it * 128:(it + 1) * 128])
                            sTlo.append(tl)

                        c_cols = []
                        for ntl in range(NTC):
                            z_ps = psz.tile([128, H], F32, tag="z_ps", name="z_ps")
                            for jh in range(H // 512):
                                zs = z_ps[:, jh * 512:(jh + 1) * 512]
                                for it in range(IT):
                                    lhs = sThi[it][:, ntl * 128:(ntl + 1) * 128]
                                    nc.tensor.matmul(
                                        zs, lhs, g_hi[it][:, jh * 512:(jh + 1) * 512],
                                        start=(it == 0), stop=False)
                                    nc.tensor.matmul(
                                        zs, lhs, g_lo[it][:, jh * 512:(jh + 1) * 512],
                                        start=False, stop=(it == IT - 1))
                            scr = nrm.tile([128, H], BF16, tag="nscr", name="nscr")
                            n2a = nrm.tile([128, 1], F32, tag="n2a", name="n2a")
                            nc.vector.scalar_tensor_tensor(
                                out=scr[:], in0=z_ps[:], scalar=1.0,
                                in1=shi_nat[ntl][:], op0=AL.mult, op1=AL.mult,
                                accum_out=n2a[:])
                            scr2 = nrm.tile([128, H], BF16, tag="nscr", name="nscr")
                            n2b = nrm.tile([128, 1], F32, tag="n2b", name="n2b")
                            nc.vector.scalar_tensor_tensor(
                                out=scr2[:], in0=z_ps[:], scalar=1.0,
                                in1=slo_nat[ntl][:], op0=AL.mult, op1=AL.mult,
                                accum_out=n2b[:])
                            n2 = nrm.tile([128, 1], F32, tag="n2", name="n2")
                            nc.vector.scalar_tensor_tensor(
                                out=n2[:], in0=n2b[:], scalar=2.0, in1=n2a[:],
                                op0=AL.mult, op1=AL.add)
                            rrec = nrm.tile([128, 1], F32, tag="rrec", name="rrec")
                            nc.vector.reciprocal(rrec[:], n2[:])
                            rk = nrm.tile([128, 1], F32, tag="rk", name="rk")
                            nc.scalar.sqrt(rk[:], rrec[:])
                            t = j * NTC + ntl
                            c_col = nrm.tile([128, 1], F32, tag="c_col", name="c_col", bufs=NTC + 1)
                            nc.vector.tensor_tensor(out=c_col[:], in0=rk[:],
                                                    in1=w2[:, t:t + 1], op=AL.mult)
                            c_cols.append(c_col)

                        cbc_ps = psmisc.tile([128, CH], F32, tag="cbc_ps", name="cbc_ps")
                        for ntl in range(NTC):
                            crow_ps = psmisc.tile([1, 128], F32, tag="crow_ps", name="crow_ps")
                            nc.tensor.transpose(crow_ps[:], c_cols[ntl][:], ident[:])
                            crow = nrm.tile([1, 128], F32, tag="crow", name="crow", bufs=2)
                            nc.scalar.copy(crow[:], crow_ps[:])
                            nc.tensor.matmul(cbc_ps[:, ntl * 128:(ntl + 1) * 128],
                                             ones_row[:], crow[:],
                                             start=True, stop=True)
                        c_bc = nrm.tile([128, CH], F32, tag="c_bc", name="c_bc")
                        nc.scalar.copy(c_bc[:], cbc_ps[:])

                        for bt in range(BT):
                            s_ps = pssim.tile([128, CH], F32, tag="s_ps", name="s_ps")
                            for it in range(IT):
                                lhs_hi = qkT_hi[it][:, bt * 128:(bt + 1) * 128]
                                lhs_lo = qkT_lo[it][:, bt * 128:(bt + 1) * 128]
                                nc.tensor.matmul(s_ps[:], lhs_hi, sThi[it][:],
                                                 start=(it == 0), stop=False)
                                nc.tensor.matmul(s_ps[:], lhs_hi, sTlo[it][:],
                                                 start=False, stop=False)
                                nc.tensor.matmul(s_ps[:], lhs_lo, sThi[it][:],
                                                 start=False, stop=(it == IT - 1))
                            scaled = simb.tile([128, CH], F32, tag="scaled", name="scaled")
                            nc.vector.tensor_tensor(out=scaled[:], in0=s_ps[:],
                                                    in1=c_bc[:], op=AL.mult)
                            vslice = vals_all[bt][:, j * 8:(j + 1) * 8]
                            nc.vector.max(vslice, scaled[:])
                            midx = simb.tile([128, 8], U32, tag="midx", name="midx")
                            nc.vector.max_index(midx[:], vslice, scaled[:])
                            midf = simb.tile([128, 8], F32, tag="midf", name="midf")
                            nc.vector.tensor_copy(midf[:], midx[:])
                            nc.vector.tensor_scalar(
                                out=idx_all[bt][:, j * 8:(j + 1) * 8], in0=midf[:],
                                scalar1=float(j * CH), scalar2=None, op0=AL.add)

                # ---- final local top-8 per query tile + pack ----
                with tc.tile_pool(name="fsel", bufs=3) as fsel:
                    for bt in range(0 if phase_stop == "prolog" else BT):
                        pack = fsel.tile([128, 16], F32, tag="pack", name="pack")
                        lvals = pack[:, 0:8]
                        nc.vector.max(lvals, vals_all[bt][:])
                        idxm = fsel.tile([128, NCH * 8], F32, tag="idxm", name="idxm")
                        nc.vector.tensor_scalar(out=idxm[:], in0=idx_all[bt][:],
                                                scalar1=BIG, scalar2=None,
                                                op0=AL.subtract)
                        lidxm = fsel.tile([128, 8], F32, tag="lidxm", name="lidxm")
                        for k in range(8):
                            mask = fsel.tile([128, NCH * 8], F32, tag="mask", name="mask")
                            nc.vector.tensor_scalar(out=mask[:], in0=vals_all[bt][:],
                                                    scalar1=lvals[:, k:k + 1],
                                                    scalar2=None, op0=AL.is_equal)
                            msel = fsel.tile([128, NCH * 8], F32, tag="msel", name="msel")
                            nc.vector.tensor_tensor(out=msel[:], in0=mask[:],
                                                    in1=idxm[:], op=AL.mult)
                            nc.vector.tensor_reduce(out=lidxm[:, k:k + 1], in_=msel[:],
                                                    axis=mybir.AxisListType.X,
                                                    op=AL.min)
                        nc.vector.tensor_scalar(out=pack[:, 8:16], in0=lidxm[:],
                                                scalar1=BIG,
                                                scalar2=nbase_bc[:, 0:1],
                                                op0=AL.add, op1=AL.add)
                        nc.sync.dma_start(pack_in[bt * 128:(bt + 1) * 128, :], pack[:])

            if phase_stop != "prolog":
                if coll:
                    nc.gpsimd.collective_compute(
                        "AllGather", AL.bypass, replica_groups=grp,
                        ins=[pack_in.opt()], outs=[pack_out.opt()])
                else:
                    for c in range(NC):
                        nc.sync.dma_start(pack_out[c * B:(c + 1) * B, :], pack_in[:])

            # ====== tail: Wv/Wo prep ‖ global select + combine; RS; projection ======
            with (
                tc.tile_pool(name="gsel", bufs=3) as gs,
                tc.tile_pool(name="wvo", bufs=1) as wvo,
                tc.tile_pool(name="comb", bufs=3) as cb,
                tc.tile_pool(name="psc", bufs=1, space="PSUM") as psc,
            ):
                wvT_hi = [wvo.tile([128, H], BF16, tag=f"wvT_hi{t}", name=f"wvT_hi{t}") for t in range(IT)]
                wvT_lo = [wvo.tile([128, H], BF16, tag=f"wvT_lo{t}", name=f"wvT_lo{t}") for t in range(IT)]
                woT_hi = [wvo.tile([128, H], BF16, tag=f"woT_hi{t}", name=f"woT_hi{t}") for t in range(IT)]
                woT_lo = [wvo.tile([128, H], BF16, tag=f"woT_lo{t}", name=f"woT_lo{t}") for t in range(IT)]
                PROJ = phase_stop == "all"
                for (src, dsthi, dstlo) in (((wv_full, wvT_hi, wvT_lo),
                                             (wo_full, woT_hi, woT_lo))
                                            if PROJ else ()):
                    for ot in range(IT):
                        wnat = cb.tile([128, H], F32, tag="wnat", name="wnat")
                        nc.sync.dma_start(wnat[:], src[ot * 128:(ot + 1) * 128, :])
                        for it in range(IT):
                            wps = psc.tile([128, 128], F32, tag="wps", name="wps")
                            nc.tensor.transpose(
                                wps[:], wnat[:, it * 128:(it + 1) * 128], ident[:])
                            dh = dsthi[it][:, ot * 128:(ot + 1) * 128]
                            dl = dstlo[it][:, ot * 128:(ot + 1) * 128]
                            nc.scalar.copy(dh, wps[:])
                            nc.vector.tensor_tensor(out=dl, in0=wps[:], in1=dh,
                                                    op=AL.subtract)

                grow_t = [gs.tile([128, H], F32, tag=f"grow{k}", name=f"grow{k}",
                                  bufs=1) for k in range(8)]
                pk3 = pack_out[:].rearrange("(cc b) k -> b cc k", cc=NC)
                for bt in range(0 if phase_stop in ("prolog", "main") else BT):
                    valsg = gs.tile([128, NC * 8], F32, tag="valsg", name="valsg")
                    idxg = gs.tile([128, NC * 8], F32, tag="idxg", name="idxg")
                    nc.sync.dma_start(valsg[:],
                                      pk3[bt * 128:(bt + 1) * 128, :, 0:8])
                    nc.sync.dma_start(idxg[:],
                                      pk3[bt * 128:(bt + 1) * 128, :, 8:16])

                    gvals = gs.tile([128, 8], F32, tag="gvals", name="gvals")
                    nc.vector.max(gvals[:], valsg[:])
                    idxm2 = gs.tile([128, NC * 8], F32, tag="idxm2", name="idxm2")
                    nc.vector.tensor_scalar(out=idxm2[:], in0=idxg[:], scalar1=BIG,
                                            scalar2=None, op0=AL.subtract)
                    gidxf = gs.tile([128, 8], F32, tag="gidxf", name="gidxf")
                    for k in range(8):
                        mask2 = gs.tile([128, NC * 8], F32, tag="mask2", name="mask2")
                        nc.vector.tensor_scalar(out=mask2[:], in0=valsg[:],
                                                scalar1=gvals[:, k:k + 1],
                                                scalar2=None, op0=AL.is_equal)
                        msel2 = gs.tile([128, NC * 8], F32, tag="msel2", name="msel2")
                        nc.vector.tensor_tensor(out=msel2[:], in0=mask2[:],
                                                in1=idxm2[:], op=AL.mult)
                        nc.vector.tensor_reduce(out=gidxf[:, k:k + 1], in_=msel2[:],
                                                axis=mybir.AxisListType.X, op=AL.min)
                    nc.vector.tensor_scalar(out=gidxf[:], in0=gidxf[:], scalar1=BIG,
                                            scalar2=None, op0=AL.add)

                    # softmax over top-8 with scale rq*rv (per query)
                    sc = gs.tile([128, 1], F32, tag="sc", name="sc")
                    nc.vector.tensor_tensor(out=sc[:], in0=rq_bt[:, bt:bt + 1],
                                            in1=rv_bc[:], op=AL.mult)
                    negm = gs.tile([128, 1], F32, tag="negm", name="negm")
                    nc.vector.scalar_tensor_tensor(out=negm[:], in0=gvals[:, 0:1],
                                                   scalar=-1.0, in1=sc[:],
                                                   op0=AL.mult, op1=AL.mult)
                    ex = gs.tile([128, 8], F32, tag="ex", name="ex")
                    nc.scalar.activation(ex[:], gvals[:], ACTF.Exp,
                                         bias=negm[:, 0:1], scale=sc[:, 0:1])
                    esum = gs.tile([128, 1], F32, tag="esum", name="esum")
                    nc.vector.tensor_reduce(out=esum[:], in_=ex[:],
                                            axis=mybir.AxisListType.X, op=AL.add)
                    esr = gs.tile([128, 1], F32, tag="esr", name="esr")
                    nc.vector.reciprocal(esr[:], esum[:])
                    attn = gs.tile([128, 8], F32, tag="attn", name="attn")
                    nc.vector.tensor_scalar(out=attn[:], in0=ex[:],
                                            scalar1=esr[:, 0:1], scalar2=None,
                                            op0=AL.mult)

                    # ownership mask + clamped local index
                    lidx = gs.tile([128, 8], F32, tag="lidx", name="lidx")
                    nc.vector.tensor_scalar(out=lidx[:], in0=gidxf[:],
                                            scalar1=nbase_bc[:, 0:1],
                                            scalar2=None, op0=AL.subtract)
                    mge = gs.tile([128, 8], F32, tag="mge", name="mge")
                    nc.vector.tensor_scalar(out=mge[:], in0=lidx[:], scalar1=0.0,
                                            scalar2=None, op0=AL.is_ge)
                    mlt = gs.tile([128, 8], F32, tag="mlt", name="mlt")
                    nc.vector.tensor_scalar(out=mlt[:], in0=lidx[:],
                                            scalar1=float(NL),
                                            scalar2=None, op0=AL.is_lt)
                    maskt = gs.tile([128, 8], F32, tag="maskt", name="maskt")
                    nc.vector.tensor_tensor(out=maskt[:], in0=mge[:], in1=mlt[:],
                                            op=AL.mult)
                    attn_m = gs.tile([128, 8], F32, tag="attn_m", name="attn_m")
                    nc.vector.tensor_tensor(out=attn_m[:], in0=maskt[:], in1=attn[:],
                                            op=AL.mult)
                    # bias non-owned indices out of range so the bounds-checked
                    # gather skips their DMA entirely (rows pre-zeroed; attn_m=0)
                    BIGIDX = 1.0e7
                    lidxb = gs.tile([128, 8], F32, tag="lidxb", name="lidxb")
                    nc.vector.tensor_scalar(out=lidxb[:], in0=lidx[:],
                                            scalar1=BIGIDX, scalar2=None,
                                            op0=AL.add)
                    lidxs = gs.tile([128, 8], F32, tag="lidxs", name="lidxs")
                    nc.vector.scalar_tensor_tensor(out=lidxs[:], in0=maskt[:],
                                                   scalar=-BIGIDX, in1=lidxb[:],
                                                   op0=AL.mult, op1=AL.add)
                    lidxu = gs.tile([128, 8], U32, tag="lidxu", name="lidxu")
                    nc.vector.tensor_copy(lidxu[:], lidxs[:])

                    comb = gs.tile([128, H], F32, tag="comb", name="comb", bufs=2)
                    for k in range(8):
                        # persistent per-k gather tiles, zeroed once: skipped
                        # (non-owned) rows then always hold 0 or stale store
                        # data, both finite, and attn_m=0 cancels them
                        grow = grow_t[k]
                        if bt == 0:
                            nc.vector.memset(grow[:], 0.0)
                        nc.gpsimd.indirect_dma_start(
                            out=grow[:], out_offset=None, in_=store_l[:],
                            in_offset=bass.IndirectOffsetOnAxis(
                                ap=lidxu[:, k:k + 1], axis=0),
                            bounds_check=NL - 1, oob_is_err=False)
                        if k == 0:
                            nc.vector.tensor_scalar(out=comb[:], in0=grow[:],
                                                    scalar1=attn_m[:, k:k + 1],
                                                    scalar2=None, op0=AL.mult)
                        else:
                            nc.vector.scalar_tensor_tensor(
                                out=comb[:], in0=grow[:], scalar=attn_m[:, k:k + 1],
                                in1=comb[:], op0=AL.mult, op1=AL.add)
                    nc.sync.dma_start(rs_in[bt * 128:(bt + 1) * 128, :], comb[:])

                if phase_stop not in ("prolog", "main"):
                    if coll:
                        nc.gpsimd.collective_compute(
                            "ReduceScatter", AL.add, replica_groups=grp,
                            ins=[rs_in.opt()], outs=[rs_out.opt()])
                    else:
                        nc.sync.dma_start(rs_out[:], rs_in[0:BSH, :])

                # ---- projection (query shard) ----
                for qt in range(QT if PROJ else 0):
                    comb = cb.tile([128, H], F32, tag="combq", name="combq")
                    nc.sync.dma_start(comb[:], rs_out[qt * 128:(qt + 1) * 128, :])

                    cT_hi = [cb.tile([128, 128], BF16, tag=f"cT_hi{t}", name=f"cT_hi{t}")
                             for t in range(IT)]
                    cT_lo = [cb.tile([128, 128], BF16, tag=f"cT_lo{t}", name=f"cT_lo{t}")
                             for t in range(IT)]
                    for it in range(IT):
                        cps = psc.tile([128, 128], F32, tag="cps", name="cps")
                        nc.tensor.transpose(cps[:], comb[:, it * 128:(it + 1) * 128],
                                            ident[:])
                        nc.scalar.copy(cT_hi[it][:], cps[:])
                        nc.vector.tensor_tensor(out=cT_lo[it][:], in0=cps[:],
                                                in1=cT_hi[it][:], op=AL.subtract)

                    y1_hi = [cb.tile([128, 128], BF16, tag=f"y1_hi{t}", name=f"y1_hi{t}")
                             for t in range(IT)]
                    y1_lo = [cb.tile([128, 128], BF16, tag=f"y1_lo{t}", name=f"y1_lo{t}")
                             for t in range(IT)]
                    for ot in range(IT):
                        yps = psc.tile([128, 128], F32, tag="yps", name="yps")
                        for it in range(IT):
                            lhs_hi = wvT_hi[it][:, ot * 128:(ot + 1) * 128]
                            lhs_lo = wvT_lo[it][:, ot * 128:(ot + 1) * 128]
                            nc.tensor.matmul(yps[:], lhs_hi, cT_hi[it][:],
                                             start=(it == 0), stop=False)
                            nc.tensor.matmul(yps[:], lhs_hi, cT_lo[it][:],
                                             start=False, stop=False)
                            nc.tensor.matmul(yps[:], lhs_lo, cT_hi[it][:],
                                             start=False, stop=(it == IT - 1))
                        nc.scalar.copy(y1_hi[ot][:], yps[:])
                        nc.vector.tensor_tensor(out=y1_lo[ot][:], in0=yps[:],
                                                in1=y1_hi[ot][:], op=AL.subtract)

                    for ot in range(IT):
                        y2ps = psc.tile([128, 128], F32, tag="y2ps", name="y2ps")
                        for it in range(IT):
                            lhs_hi = woT_hi[it][:, ot * 128:(ot + 1) * 128]
                            lhs_lo = woT_lo[it][:, ot * 128:(ot + 1) * 128]
                            nc.tensor.matmul(y2ps[:], lhs_hi, y1_hi[it][:],
                                             start=(it == 0), stop=False)
                            nc.tensor.matmul(y2ps[:], lhs_hi, y1_lo[it][:],
                                             start=False, stop=False)
                            nc.tensor.matmul(y2ps[:], lhs_lo, y1_hi[it][:],
                                             start=False, stop=(it == IT - 1))
                        y2sb = cb.tile([128, 128], F32, tag="y2sb", name="y2sb")
                        nc.scalar.copy(y2sb[:], y2ps[:])
                        yout_ps = psc.tile([128, 128], F32, tag="yout_ps", name="yout_ps")
                        nc.tensor.transpose(yout_ps[:], y2sb[:], ident[:])
                        yout = cb.tile([128, 128], F32, tag="yout", name="yout")
                        nc.scalar.copy(yout[:], yout_ps[:])
                        nc.sync.dma_start(
                            out_d[qt * 128:(qt + 1) * 128,
                                  ot * 128:(ot + 1) * 128],
                            yout[:])

    nc.compile()
    return nc


_CACHE = {}


def _get_nc(B, N, H, NC):
    key = (B, N, H, NC)
    if key not in _CACHE:
        _CACHE[key] = build_kernel(B, N, H, NC)
    return _CACHE[key]


def make_in_maps(query, store, importance, timestamps, Wk, Wv, Wo, NC=8):
    B, H = query.shape
    N = store.shape[0]
    NL, BSH, WSH = N // NC, B // NC, H // NC
    in_maps = []
    for c in range(NC):
        in_maps.append({
            "store_l": store[c * NL:(c + 1) * NL],
            "imp_l": importance[c * NL:(c + 1) * NL],
            "ts_l": timestamps[c * NL:(c + 1) * NL],
            "q_sh": query[c * BSH:(c + 1) * BSH],
            "wk_sh": Wk[c * WSH:(c + 1) * WSH],
            "wv_sh": Wv[c * WSH:(c + 1) * WSH],
            "wo_sh": Wo[c * WSH:(c + 1) * WSH],
            "nbase_d": np.array([[c * NL]], dtype=np.float32),
        })
    return in_maps


def kernel(query, store, importance, timestamps, Wk, Wv, Wo):
    query = np.ascontiguousarray(np.asarray(query, dtype=np.float32))
    store = np.ascontiguousarray(np.asarray(store, dtype=np.float32))
    importance = np.ascontiguousarray(np.asarray(importance, dtype=np.float32))
    timestamps = np.ascontiguousarray(np.asarray(timestamps, dtype=np.float32))
    Wk = np.ascontiguousarray(np.asarray(Wk, dtype=np.float32))
    Wv = np.ascontiguousarray(np.asarray(Wv, dtype=np.float32))
    Wo = np.ascontiguousarray(np.asarray(Wo, dtype=np.float32))

    B, H = query.shape
    N = store.shape[0]
    NC = 8
    nc = _get_nc(B, N, H, NC)
    in_maps = make_in_maps(query, store, importance, timestamps, Wk, Wv, Wo, NC)
    res = run_bass_kernel_spmd(nc, in_maps, core_ids=list(range(NC)))
    out = np.concatenate([res.results[c]["out_shard"] for c in range(NC)], axis=0)
    return np.ascontiguousarray(out, dtype=np.float32)
